# revision 20
# baseline (speedup 1.0000x reference)
"""Bass/Tile kernel: cosine top-20 adjacency (16384x64 embeddings) on 8 trn2 cores.

Per-core algorithm (rows sharded 2048/core via host-side input rotation, so the
same SPMD graph runs on every core):
  1. Load embeddings row-major, compute row norms (square -> windowed reduce ->
     sqrt -> reciprocal), fused normalize+bf16-cast.
  2. Round-trip through DRAM and XBAR-transpose the [8192, 128] bf16 view ->
     nt2 [128, 8192]: partition p<64 holds dim p of EVEN local rows, p>=64
     holds dim p-64 of ODD local rows (column m = local row 2m / 2m+1).
  3. Main loop over 16 row tiles of 128 rows (t<8 even rows, t>=8 odd rows).
     The K=64 contraction uses only half the PE array, so matmuls are issued
     in pairs on disjoint PE row-groups (tile_position (0,0) / (64,0)): the
     rg0 matmul computes sims against EVEN columns (rhs = nt2[0:64]), the
     rg64 matmul against ODD columns (rhs = nt2[64:128]) -- they run
     concurrently, doubling tensor throughput. Output is values-only so the
     column permutation is irrelevant. lhsT mirrors (mir) provide each tile's
     weights on the opposite partition half.
  4. Per 2048-col PSUM group, evacuation is split between engines
     (GROUP_KIND): "A" = Act copies f32->bf16; "D" = DVE tensor_max folds
     PSUM halves to 1024 bf16. A flat fold tree reduces everything to 512
     windowed maxima, max8 per 128-chunk -> 32 candidates, 3x(max8 +
     match_replace) -> top-24 descending.
  5. Self-similarity (~1.0) is always the strict row max, so
     out[:,0] = 0 and out[:,1:20] = sigmoid(top24[:,1:20]).
"""

import os
import sys

import numpy as np

for _p in ("/opt/trn_rl_repo",):
    if _p not in sys.path and os.path.isdir(_p):
        sys.path.insert(0, _p)

import concourse.bass as bass  # noqa: E402
import concourse.mybir as mybir  # noqa: E402
import concourse.tile as tile  # noqa: E402
from concourse import bacc  # noqa: E402
from concourse.bass_utils import run_bass_kernel_spmd  # noqa: E402

N = 16384
D = 64
TOPK = 20
CORES = 8
R = N // CORES  # 2048 rows per core
T = R // 128  # 16 row tiles per core
G = 2048  # PSUM column group size (sim values per group)
NG = N // G  # 8 groups per tile
H = N // 2  # 8192: even/odd half size (nt2 free dim)
NEG = -1.0e30

f32 = mybir.dt.float32
bf16 = mybir.dt.bfloat16
AF = mybir.ActivationFunctionType
ALU = mybir.AluOpType

# Per-group evacuation: "A" = Act copies all 2048 f32->bf16 (folded later by
# the DVE tree at bf16 2x), "D" = DVE evacuates via a chained fold
# (copy ps_lo, then max(ps_hi, prev) -- only one PSUM operand per op, per
# NCC_IBVF027), which relieves Act and folds into a single 1024 vector.
GROUP_KIND = ("A", "D", "A", "A", "A", "D", "A", "A")

_CACHE = {}


def _build_nc():
    nc = bacc.Bacc(
        "TRN2", target_bir_lowering=False, debug=False, enable_asserts=False
    )
    emb = nc.dram_tensor("embeddings", [N, D], f32, kind="ExternalInput")
    out = nc.dram_tensor("out", [R, TOPK], f32, kind="ExternalOutput")
    # tile t<8 covers even local rows 2*(t*128+q), t>=8 covers odd rows
    # 2*((t-8)*128+q)+1. This view un-permutes on the output DMA:
    # out_v[h, j] = local row 2j+h.
    out_v = out[:].rearrange("(j two) k -> two j k", two=2)

    nA = GROUP_KIND.count("A")
    nD = GROUP_KIND.count("D")

    with tile.TileContext(nc) as tc:
        with tc.tile_pool(name="persist", bufs=1) as persist:
            # nt2[p<64, m] = dim p of local row 2m; nt2[64+p, m] = row 2m+1
            nt2 = persist.tile([128, H], bf16)
            # lhsT mirrors: mir[0:64, s] = odd-row dims (for rg0 matmuls of
            # odd tiles), mir[64:128, s] = even-row dims (for rg64 matmuls of
            # even tiles). s in [0, 1024) spans the core's own 2048 rows.
            mir = persist.tile([128, R // 2], bf16)

            # ---- Prologue: normalize rows, cast bf16, XBAR transpose ----
            with (
                tc.tile_pool(name="pro_rm", bufs=1) as pro_rm,
                tc.tile_pool(name="pro_dram", bufs=1, space="DRAM") as pro_dram,
            ):
                # flat [128, 128, 64] staging view: row r = p*128 + a
                # (contiguous 8 KB per-partition DMA runs; an a-major layout
                # would fragment the DMA into 256 B bursts and stall the
                # whole norm pipeline behind slow loads).
                emb_v = emb[:].rearrange("(p a) d -> p a d", p=128)
                rm = pro_rm.tile([128, 128, D], f32)
                sq = pro_rm.tile([128, 128, D], f32)
                ssq = pro_rm.tile([128, 128], f32)
                slen = pro_rm.tile([128, 128], f32)
                sinv = pro_rm.tile([128, 128], f32)
                rmb = pro_rm.tile([128, 128, D], bf16)
                scratch = pro_dram.tile([N, D], bf16)
                sc_v = scratch[:].rearrange("(p a) d -> p a d", p=128)
                engs = (nc.sync, nc.scalar, nc.sync, nc.scalar)
                NCH = 4
                CW = 128 // NCH
                for c in range(NCH):
                    cs = slice(c * CW, (c + 1) * CW)
                    engs[c].dma_start(rm[:, cs, :], emb_v[:, cs, :])
                    nc.scalar.activation(sq[:, cs, :], rm[:, cs, :], AF.Square)
                    nc.vector.tensor_reduce(
                        ssq[:, cs], sq[:, cs, :],
                        axis=mybir.AxisListType.X, op=ALU.add,
                    )
                    nc.scalar.activation(slen[:, cs], ssq[:, cs], AF.Sqrt)
                    nc.vector.reciprocal(sinv[:, cs], slen[:, cs])
                    nc.vector.scalar_tensor_tensor(
                        rmb[:, cs, :], rm[:, cs, :], 1.0,
                        sinv[:, cs].to_broadcast((128, CW, D)),
                        op0=ALU.mult, op1=ALU.mult,
                    )
                    engs[c].dma_start(sc_v[:, cs, :], rmb[:, cs, :])

                # XBAR transpose of the [8192, 128] bf16 view in two row
                # chunks, BOTH on the scalar ring (concurrent transposes on
                # two rings corrupt the XBAR; smaller chunks pay too much
                # per-transpose overhead).
                sc_t = scratch[:].rearrange("(m two) d -> m (two d)", two=2)
                for c in range(2):
                    ms = slice(c * 4096, (c + 1) * 4096)
                    nc.scalar.dma_start(
                        out=nt2[:, ms], in_=sc_t[ms, :], transpose=True
                    )
                # lhsT mirrors via partition-shift SBUF->SBUF DMAs (128 KB);
                # they only need transpose chunk 0 (m < 1024).
                nc.sync.dma_start(mir[0:64, :], nt2[64:128, 0 : R // 2])
                nc.sync.dma_start(mir[64:128, :], nt2[0:64, 0 : R // 2])

            # ---- Main loop: 16 row tiles ----
            with (
                tc.tile_pool(name="mm_psum", bufs=2, space="PSUM") as mm_psum,
                tc.tile_pool(name="ev_ca", bufs=2) as ev_ca,
                tc.tile_pool(name="ev_dd", bufs=2) as ev_dd,
                tc.tile_pool(name="pyr", bufs=2) as pyr,
                tc.tile_pool(name="fin", bufs=2) as fin,
            ):
                for t in range(T):
                    if t < 8:
                        s0 = t * 128
                        lhsA = nt2[0:64, s0 : s0 + 128]
                        lhsB = mir[64:128, s0 : s0 + 128]
                    else:
                        s0 = (t - 8) * 128
                        lhsA = mir[0:64, s0 : s0 + 128]
                        lhsB = nt2[64:128, s0 : s0 + 128]

                    ca = ev_ca.tile([128, nA, G], bf16, tag="ca")
                    # chained D-group evacuation: dch[:, i] for i = 0..2*nD-1
                    dch = ev_dd.tile([128, 2 * nD, G // 2], bf16, tag="dch")
                    iA = 0
                    iD = 0
                    for g in range(NG):
                        ps = mm_psum.tile([128, G], f32, tag="ps")
                        # 4 matmuls alternating PE row-groups; m-range
                        # [g*1024, (g+1)*1024) x {even, odd} columns.
                        for s in range(2):
                            ms = slice(g * 1024 + s * 512, g * 1024 + (s + 1) * 512)
                            nc.tensor.matmul(
                                ps[:, s * 1024 : s * 1024 + 512],
                                lhsA,
                                nt2[0:64, ms],
                            )
                            nc.tensor.matmul(
                                ps[:, s * 1024 + 512 : s * 1024 + 1024],
                                lhsB,
                                nt2[64:128, ms],
                            )
                        if GROUP_KIND[g] == "A":
                            nc.scalar.activation(
                                ca[:, iA, :], ps[:], AF.Copy
                            )
                            iA += 1
                        else:
                            if iD == 0:
                                nc.vector.tensor_copy(
                                    dch[:, 0, :], ps[:, 0 : G // 2]
                                )
                            else:
                                nc.vector.tensor_max(
                                    dch[:, 2 * iD, :],
                                    ps[:, 0 : G // 2],
                                    dch[:, 2 * iD - 1, :],
                                )
                            nc.vector.tensor_max(
                                dch[:, 2 * iD + 1, :],
                                ps[:, G // 2 : G],
                                dch[:, 2 * iD, :],
                            )
                            iD += 1

                    # Flat fold tree: 6x2048 (ca) + 1024 (dch tail) -> 512.
                    w0 = pyr.tile([128, 3, G], bf16, tag="w0")
                    nc.vector.tensor_max(w0[:], ca[:, 0:3, :], ca[:, 3:6, :])
                    w1 = pyr.tile([128, G], bf16, tag="w1")
                    nc.vector.tensor_max(w1[:], w0[:, 0, :], w0[:, 1, :])
                    w2 = pyr.tile([128, G], bf16, tag="w2")
                    nc.vector.tensor_max(w2[:], w1[:], w0[:, 2, :])
                    w3 = pyr.tile([128, G // 2], bf16, tag="w3")
                    nc.vector.tensor_max(
                        w3[:], w2[:, 0 : G // 2], w2[:, G // 2 : G]
                    )
                    w5 = pyr.tile([128, G // 2], bf16, tag="w5")
                    nc.vector.tensor_max(w5[:], w3[:], dch[:, 2 * nD - 1, :])
                    f5 = pyr.tile([128, G // 4], bf16, tag="f5")
                    nc.vector.tensor_max(
                        f5[:], w5[:, 0 : G // 4], w5[:, G // 4 : G // 2]
                    )

                    # candidates: top-8 of each 128-chunk of the 512 maxima
                    cand = fin.tile([128, 32], bf16, tag="cand")
                    for c in range(4):
                        nc.vector.max(
                            out=cand[:, c * 8 : (c + 1) * 8],
                            in_=f5[:, c * 128 : (c + 1) * 128],
                        )
                    # top-24 via 3x max8 + 2x match_replace
                    top24 = fin.tile([128, 24], bf16, tag="top24")
                    cand2 = fin.tile([128, 32], bf16, tag="cand2")
                    cand3 = fin.tile([128, 32], bf16, tag="cand3")
                    nc.vector.max(out=top24[:, 0:8], in_=cand[:])
                    nc.vector.match_replace(
                        out=cand2[:], in_to_replace=top24[:, 0:8],
                        in_values=cand[:], imm_value=NEG,
                    )
                    nc.vector.max(out=top24[:, 8:16], in_=cand2[:])
                    nc.vector.match_replace(
                        out=cand3[:], in_to_replace=top24[:, 8:16],
                        in_values=cand2[:], imm_value=NEG,
                    )
                    nc.vector.max(out=top24[:, 16:24], in_=cand3[:])

                    # epilogue: out[:,1:20] = sigmoid(top24[:,1:20]); column 0
                    # (the masked self-loop) is zeroed host-side after gather.
                    osb = fin.tile([128, TOPK - 1], f32, tag="osb")
                    nc.scalar.activation(
                        osb[:], top24[:, 1:TOPK], AF.Sigmoid
                    )
                    hh, band = (0, t) if t < 8 else (1, t - 8)
                    nc.sync.dma_start(
                        out_v[hh, band * 128 : (band + 1) * 128, 1:TOPK],
                        osb[:],
                    )

    nc.compile()
    return nc


def get_nc():
    if "nc" not in _CACHE:
        _CACHE["nc"] = _build_nc()
    return _CACHE["nc"]


def kernel(embeddings: np.ndarray) -> np.ndarray:
    emb = np.ascontiguousarray(np.asarray(embeddings, dtype=np.float32))
    assert emb.shape == (N, D), emb.shape
    nc = get_nc()
    in_maps = [
        {"embeddings": np.roll(emb, -i * R, axis=0)} for i in range(CORES)
    ]
    res = run_bass_kernel_spmd(nc, in_maps, core_ids=list(range(CORES)))
    _CACHE["last_results"] = res
    full = np.concatenate(
        [res.results[i]["out"] for i in range(CORES)], axis=0
    ).astype(np.float32)
    full[:, 0] = 0.0  # masked self-loop column (never written by the kernel)
    return full


# revision 23
# speedup vs baseline: 1.0113x; 1.0113x over previous
"""Bass/Tile kernel: cosine top-20 adjacency (16384x64 embeddings) on 8 trn2 cores.

Per-core algorithm (rows sharded 2048/core via host-side input rotation, so the
same SPMD graph runs on every core):
  1. Load embeddings row-major, compute row norms (square -> windowed reduce ->
     sqrt -> reciprocal), fused normalize+bf16-cast.
  2. Round-trip through DRAM and XBAR-transpose the [8192, 128] bf16 view ->
     nt2 [128, 8192]: partition p<64 holds dim p of EVEN local rows, p>=64
     holds dim p-64 of ODD local rows (column m = local row 2m / 2m+1).
  3. Main loop over 16 row tiles of 128 rows (t<8 even rows, t>=8 odd rows).
     The K=64 contraction uses only half the PE array, so matmuls are issued
     in pairs on disjoint PE row-groups (tile_position (0,0) / (64,0)): the
     rg0 matmul computes sims against EVEN columns (rhs = nt2[0:64]), the
     rg64 matmul against ODD columns (rhs = nt2[64:128]) -- they run
     concurrently, doubling tensor throughput. Output is values-only so the
     column permutation is irrelevant. lhsT mirrors (mir) provide each tile's
     weights on the opposite partition half.
  4. Per 2048-col PSUM group, evacuation is split between engines
     (GROUP_KIND): "A" = Act copies f32->bf16; "D" = DVE tensor_max folds
     PSUM halves to 1024 bf16. A flat fold tree reduces everything to 512
     windowed maxima, max8 per 128-chunk -> 32 candidates, 3x(max8 +
     match_replace) -> top-24 descending.
  5. Self-similarity (~1.0) is always the strict row max, so
     out[:,0] = 0 and out[:,1:20] = sigmoid(top24[:,1:20]).
"""

import os
import sys

import numpy as np

for _p in ("/opt/trn_rl_repo",):
    if _p not in sys.path and os.path.isdir(_p):
        sys.path.insert(0, _p)

import concourse.bass as bass  # noqa: E402
import concourse.mybir as mybir  # noqa: E402
import concourse.tile as tile  # noqa: E402
from concourse import bacc  # noqa: E402
from concourse.bass_utils import run_bass_kernel_spmd  # noqa: E402

N = 16384
D = 64
TOPK = 20
CORES = 8
R = N // CORES  # 2048 rows per core
T = R // 128  # 16 row tiles per core
G = 2048  # PSUM column group size (sim values per group)
NG = N // G  # 8 groups per tile
H = N // 2  # 8192: even/odd half size (nt2 free dim)
NEG = -1.0e30

f32 = mybir.dt.float32
bf16 = mybir.dt.bfloat16
AF = mybir.ActivationFunctionType
ALU = mybir.AluOpType

# Per-group evacuation: "A" = Act copies all 2048 f32->bf16 (folded later by
# the DVE tree at bf16 2x), "D" = DVE evacuates via a chained fold
# (copy ps_lo, then max(ps_hi, prev) -- only one PSUM operand per op, per
# NCC_IBVF027), which relieves Act and folds into a single 1024 vector.
GROUP_KIND = ("A", "D", "A", "A", "A", "D", "A", "A")

_CACHE = {}


def _build_nc():
    nc = bacc.Bacc(
        "TRN2", target_bir_lowering=False, debug=False, enable_asserts=False
    )
    emb = nc.dram_tensor("embeddings", [N, D], f32, kind="ExternalInput")
    out = nc.dram_tensor("out", [R, TOPK], f32, kind="ExternalOutput")
    # tile t<8 covers even local rows 2*(t*128+q), t>=8 covers odd rows
    # 2*((t-8)*128+q)+1. This view un-permutes on the output DMA:
    # out_v[h, j] = local row 2j+h.
    out_v = out[:].rearrange("(j two) k -> two j k", two=2)

    nA = GROUP_KIND.count("A")
    nD = GROUP_KIND.count("D")

    with tile.TileContext(nc) as tc:
        with tc.tile_pool(name="persist", bufs=1) as persist:
            # nt2[p<64, m] = dim p of local row 2m; nt2[64+p, m] = row 2m+1
            nt2 = persist.tile([128, H], bf16)
            # lhsT mirrors: mir[0:64, s] = odd-row dims (for rg0 matmuls of
            # odd tiles), mir[64:128, s] = even-row dims (for rg64 matmuls of
            # even tiles). s in [0, 1024) spans the core's own 2048 rows.
            mir = persist.tile([128, R // 2], bf16)

            # ---- Prologue: normalize rows, cast bf16, XBAR transpose ----
            with (
                tc.tile_pool(name="pro_rm", bufs=1) as pro_rm,
                tc.tile_pool(name="pro_dram", bufs=1, space="DRAM") as pro_dram,
            ):
                # flat [128, 128, 64] staging view: row r = p*128 + a
                # (contiguous 8 KB per-partition DMA runs; an a-major layout
                # would fragment the DMA into 256 B bursts and stall the
                # whole norm pipeline behind slow loads).
                emb_v = emb[:].rearrange("(p a) d -> p a d", p=128)
                rm = pro_rm.tile([128, 128, D], f32)
                sq = pro_rm.tile([128, 128, D], f32)
                ssq = pro_rm.tile([128, 128], f32)
                slen = pro_rm.tile([128, 128], f32)
                sinv = pro_rm.tile([128, 128], f32)
                rmb = pro_rm.tile([128, 128, D], bf16)
                scratch = pro_dram.tile([N, D], bf16)
                sc_v = scratch[:].rearrange("(p a) d -> p a d", p=128)
                engs = (nc.sync, nc.scalar, nc.sync, nc.scalar)
                NCH = 4
                CW = 128 // NCH
                for c in range(NCH):
                    cs = slice(c * CW, (c + 1) * CW)
                    engs[c].dma_start(rm[:, cs, :], emb_v[:, cs, :])
                    nc.scalar.activation(sq[:, cs, :], rm[:, cs, :], AF.Square)
                    nc.vector.tensor_reduce(
                        ssq[:, cs], sq[:, cs, :],
                        axis=mybir.AxisListType.X, op=ALU.add,
                    )
                    nc.scalar.activation(slen[:, cs], ssq[:, cs], AF.Sqrt)
                    nc.vector.reciprocal(sinv[:, cs], slen[:, cs])
                    nc.vector.scalar_tensor_tensor(
                        rmb[:, cs, :], rm[:, cs, :], 1.0,
                        sinv[:, cs].to_broadcast((128, CW, D)),
                        op0=ALU.mult, op1=ALU.mult,
                    )
                    engs[c].dma_start(sc_v[:, cs, :], rmb[:, cs, :])

                # XBAR transpose of the [8192, 128] bf16 view in two row
                # chunks, BOTH on the scalar ring (concurrent transposes on
                # two rings corrupt the XBAR; smaller chunks pay too much
                # per-transpose overhead).
                sc_t = scratch[:].rearrange("(m two) d -> m (two d)", two=2)
                for c in range(2):
                    ms = slice(c * 4096, (c + 1) * 4096)
                    nc.scalar.dma_start(
                        out=nt2[:, ms], in_=sc_t[ms, :], transpose=True
                    )
                # lhsT mirrors via partition-shift SBUF->SBUF DMAs (128 KB);
                # they only need transpose chunk 0 (m < 1024).
                nc.sync.dma_start(mir[0:64, :], nt2[64:128, 0 : R // 2])
                nc.sync.dma_start(mir[64:128, :], nt2[0:64, 0 : R // 2])

            # ---- Main loop: 16 row tiles ----
            with (
                # ca/dch live two tile-cycles under the pipelined emission
                # (written in iteration t, folded in iteration t+1), so give
                # them a third buffer to keep tile t+1's evacuation from
                # waiting on tile t-1's tree reads.
                tc.tile_pool(name="mm_psum", bufs=2, space="PSUM") as mm_psum,
                tc.tile_pool(name="ev_ca", bufs=3) as ev_ca,
                tc.tile_pool(name="ev_dd", bufs=3) as ev_dd,
                tc.tile_pool(name="pyr", bufs=2) as pyr,
                tc.tile_pool(name="fin", bufs=2) as fin,
            ):
                def emit_reduce(ca, dch, t):
                    # Fold tree + top-24 finals + epilogue for tile t.
                    # Emitted one iteration LATE (software pipelining): when
                    # the DVE reaches this tree its inputs are already
                    # complete, instead of stalling on tile t's last Act copy.
                    w0 = pyr.tile([128, 3, G], bf16, tag="w0")
                    nc.vector.tensor_max(w0[:], ca[:, 0:3, :], ca[:, 3:6, :])
                    w1 = pyr.tile([128, G], bf16, tag="w1")
                    nc.vector.tensor_max(w1[:], w0[:, 0, :], w0[:, 1, :])
                    w2 = pyr.tile([128, G], bf16, tag="w2")
                    nc.vector.tensor_max(w2[:], w1[:], w0[:, 2, :])
                    w3 = pyr.tile([128, G // 2], bf16, tag="w3")
                    nc.vector.tensor_max(
                        w3[:], w2[:, 0 : G // 2], w2[:, G // 2 : G]
                    )
                    w5 = pyr.tile([128, G // 2], bf16, tag="w5")
                    nc.vector.tensor_max(w5[:], w3[:], dch[:, 2 * nD - 1, :])
                    f5 = pyr.tile([128, G // 4], bf16, tag="f5")
                    nc.vector.tensor_max(
                        f5[:], w5[:, 0 : G // 4], w5[:, G // 4 : G // 2]
                    )

                    # candidates: top-8 of each 128-chunk of the 512 maxima
                    cand = fin.tile([128, 32], bf16, tag="cand")
                    for c in range(4):
                        nc.vector.max(
                            out=cand[:, c * 8 : (c + 1) * 8],
                            in_=f5[:, c * 128 : (c + 1) * 128],
                        )
                    # top-24 via 3x max8 + 2x match_replace
                    top24 = fin.tile([128, 24], bf16, tag="top24")
                    cand2 = fin.tile([128, 32], bf16, tag="cand2")
                    cand3 = fin.tile([128, 32], bf16, tag="cand3")
                    nc.vector.max(out=top24[:, 0:8], in_=cand[:])
                    nc.vector.match_replace(
                        out=cand2[:], in_to_replace=top24[:, 0:8],
                        in_values=cand[:], imm_value=NEG,
                    )
                    nc.vector.max(out=top24[:, 8:16], in_=cand2[:])
                    nc.vector.match_replace(
                        out=cand3[:], in_to_replace=top24[:, 8:16],
                        in_values=cand2[:], imm_value=NEG,
                    )
                    nc.vector.max(out=top24[:, 16:24], in_=cand3[:])

                    # epilogue: out[:,1:20] = sigmoid(top24[:,1:20]); column 0
                    # (the masked self-loop) is zeroed host-side after gather.
                    osb = fin.tile([128, TOPK - 1], f32, tag="osb")
                    nc.scalar.activation(
                        osb[:], top24[:, 1:TOPK], AF.Sigmoid
                    )
                    hh, band = (0, t) if t < 8 else (1, t - 8)
                    nc.sync.dma_start(
                        out_v[hh, band * 128 : (band + 1) * 128, 1:TOPK],
                        osb[:],
                    )

                prev = None
                for t in range(T):
                    if t < 8:
                        s0 = t * 128
                        lhsA = nt2[0:64, s0 : s0 + 128]
                        lhsB = mir[64:128, s0 : s0 + 128]
                    else:
                        s0 = (t - 8) * 128
                        lhsA = mir[0:64, s0 : s0 + 128]
                        lhsB = nt2[64:128, s0 : s0 + 128]

                    ca = ev_ca.tile([128, nA, G], bf16, tag="ca")
                    # chained D-group evacuation: dch[:, i] for i = 0..2*nD-1
                    dch = ev_dd.tile([128, 2 * nD, G // 2], bf16, tag="dch")
                    iA = 0
                    iD = 0
                    for g in range(NG):
                        ps = mm_psum.tile([128, G], f32, tag="ps")
                        # 4 matmuls alternating PE row-groups; m-range
                        # [g*1024, (g+1)*1024) x {even, odd} columns.
                        for s in range(2):
                            ms = slice(g * 1024 + s * 512, g * 1024 + (s + 1) * 512)
                            nc.tensor.matmul(
                                ps[:, s * 1024 : s * 1024 + 512],
                                lhsA,
                                nt2[0:64, ms],
                            )
                            nc.tensor.matmul(
                                ps[:, s * 1024 + 512 : s * 1024 + 1024],
                                lhsB,
                                nt2[64:128, ms],
                            )
                        if GROUP_KIND[g] == "A":
                            nc.scalar.activation(
                                ca[:, iA, :], ps[:], AF.Copy
                            )
                            iA += 1
                        else:
                            if iD == 0:
                                nc.vector.tensor_copy(
                                    dch[:, 0, :], ps[:, 0 : G // 2]
                                )
                            else:
                                nc.vector.tensor_max(
                                    dch[:, 2 * iD, :],
                                    ps[:, 0 : G // 2],
                                    dch[:, 2 * iD - 1, :],
                                )
                            nc.vector.tensor_max(
                                dch[:, 2 * iD + 1, :],
                                ps[:, G // 2 : G],
                                dch[:, 2 * iD, :],
                            )
                            iD += 1

                    if prev is not None:
                        emit_reduce(*prev)
                    prev = (ca, dch, t)
                emit_reduce(*prev)

    nc.compile()
    return nc


def get_nc():
    if "nc" not in _CACHE:
        _CACHE["nc"] = _build_nc()
    return _CACHE["nc"]


def kernel(embeddings: np.ndarray) -> np.ndarray:
    emb = np.ascontiguousarray(np.asarray(embeddings, dtype=np.float32))
    assert emb.shape == (N, D), emb.shape
    nc = get_nc()
    in_maps = [
        {"embeddings": np.roll(emb, -i * R, axis=0)} for i in range(CORES)
    ]
    res = run_bass_kernel_spmd(nc, in_maps, core_ids=list(range(CORES)))
    _CACHE["last_results"] = res
    full = np.concatenate(
        [res.results[i]["out"] for i in range(CORES)], axis=0
    ).astype(np.float32)
    full[:, 0] = 0.0  # masked self-loop column (never written by the kernel)
    return full
